# revision 35
# baseline (speedup 1.0000x reference)
"""MoE layer (top-2 of 8 experts + shared expert) as a Bass/Tile kernel on 8 TRN2 cores.

Strategy (expert parallelism, per the sharding hint):
  - Host computes the tiny gating network (softmax -> top-2 -> renormalize) and
    builds the all-to-all token dispatch: core e receives the tokens routed to
    expert e, pre-transposed to [d_model, C].
  - SPMD means every core executes the same instruction count, so the routed
    capacity is a compiled constant CP. Tokens beyond CP on overloaded experts
    ("spill") are computed as quarter-d_ff jobs spread over 4 cores each, so CP
    can sit at the mean expert load instead of the max (load balancing without
    duplicating full expert weights: a quarter weight set is 3MB).
  - Core e runs expert e's FFN on its tokens (silu(x@W1 * x@W3) @ W2), scales
    each output token by its gate weight, runs a 512-token slice of the shared
    expert (token-parallel across the 8 cores), and one spill quarter-job.
  - Host scatter-adds the expert/spill contributions per token and the shared
    output back into the full [T, d] result.

Device schedule: the shared expert runs FIRST (its weights stream in small
chunks, so the PE starts within ~2us), while the expert-path weights prefetch
in small chunks threaded through the stream queue slots; the expert path then
runs entirely from SBUF-resident weights. The spill job runs last (its tiny
epilogue minimizes the post-matmul drain tail).
"""

import os
import sys

for _p in ("/opt/trn_rl_repo",):
    if _p not in sys.path and os.path.isdir(_p):
        sys.path.insert(0, _p)

import numpy as np
import ml_dtypes

import concourse.bass as bass
import concourse.mybir as mybir
import concourse.tile as tile
from concourse import bacc
from concourse.bass_utils import run_bass_kernel_spmd


def install_ntff_hook():
    """This image's antenv lacks axon_hooks, which run_bass_kernel_spmd imports
    unconditionally when tracing; provide it and register the ctypes NTFF
    profile hook so trace=True (or BASS_TRACE=1) works."""
    import types

    try:
        import antenv.axon_hooks  # noqa: F401
        return
    except ImportError:
        pass
    mod = types.ModuleType("antenv.axon_hooks")
    _hook = [None]
    mod.set_axon_ntff_profile_hook = lambda h: _hook.__setitem__(0, h)
    mod.get_axon_ntff_profile_hook = lambda: _hook[0]
    sys.modules["antenv.axon_hooks"] = mod
    try:
        import antenv

        antenv.axon_hooks = mod
    except ImportError:
        pass
    try:
        from trn_agent_boot.trn_boot import _ntff_profile_via_ctypes

        mod.set_axon_ntff_profile_hook(
            _ntff_profile_via_ctypes("/opt/axon/libaxon_pjrt.so")
        )
    except Exception:
        pass


install_ntff_hook()

P = 128
D = 1024          # d_model
F = 2048          # d_ff per expert
FS = 4096         # shared expert hidden
E = 8             # experts == cores
TOPK = 2
TS = 512          # shared-expert tokens per core (T / 8)
T = 4096
CS = 64           # spill-job token capacity per core
NQ = 4            # spill splits d_ff into NQ quarter-jobs
FH = F // NQ      # 512: spill-job hidden width

DT = D // P       # 8
FT = F // P       # 16
FST = FS // P     # 32
FHT = FH // P     # 4
WG = 2 * P        # ws1/ws3 streaming group width

f32 = mybir.dt.float32
bf16 = mybir.dt.bfloat16

MM_CFG = os.environ.get("MOE_MM_CFG", "bf16")

_COMPILED: dict = {}


def _np_mm_dtype(cfg):
    return ml_dtypes.bfloat16 if cfg == "bf16" else np.float32


def _chunks(C):
    """Split C token columns into near-equal matmul-N chunks of <=512."""
    n = -(-C // 512)
    out = []
    s = 0
    for i in range(n):
        if i < n - 1:
            w = (-(-(C - s) // (n - i)) + 7) // 8 * 8
        else:
            w = C - s
        out.append((s, w))
        s += w
    return out


def plan_spill(counts):
    """Pick the compiled primary capacity CP and the spill quarter-jobs.

    Every token-expert pair beyond CP on an expert is computed as NQ
    quarter-d_ff jobs on NQ different cores (each core has one CS-column spill
    slot). Returns (CP, jobs) with jobs a list of (expert, quarter,
    token_start, width), len(jobs) <= E.
    """
    CP = max(8, int(-(-int(np.sum(counts)) // E // 8) * 8))
    while True:
        jobs = []
        for e in range(E):
            O = int(counts[e]) - CP
            s = CP
            while O > 0:
                w = min(CS, O)
                for q in range(NQ):
                    jobs.append((e, q, s, w))
                O -= w
                s += w
        if len(jobs) <= E:
            return CP, jobs
        CP += 8


def build_program(CP: int, cfg: str):
    """Build the per-core Bass program for primary capacity CP (+CS spill)."""
    assert cfg == "bf16"
    assert CP % 8 == 0
    sdt = bf16

    nc = bacc.Bacc("TRN2", target_bir_lowering=False, debug=False, num_devices=E)

    # ---- per-core inputs ----
    # All inputs arrive host-pretiled in the exact SBUF layout (partition
    # dim first) so every DMA is one contiguous per-partition segment.
    xgT = nc.dram_tensor("xgT", [P, DT, CP], sdt, kind="ExternalInput")
    gw = nc.dram_tensor("gw", [1, CP], sdt, kind="ExternalInput")
    w1 = nc.dram_tensor("w1", [P, DT, F], sdt, kind="ExternalInput")
    w3 = nc.dram_tensor("w3", [P, DT, F], sdt, kind="ExternalInput")
    w2 = nc.dram_tensor("w2", [P, FT, D], sdt, kind="ExternalInput")
    b1 = nc.dram_tensor("b1", [P, F // P], f32, kind="ExternalInput")
    b3 = nc.dram_tensor("b3", [P, F // P], f32, kind="ExternalInput")
    b2 = nc.dram_tensor("b2", [P, D // P], f32, kind="ExternalInput")
    xsT = nc.dram_tensor("xsT", [P, DT, TS], sdt, kind="ExternalInput")
    ws1 = nc.dram_tensor("ws1", [FS // WG, P, DT, WG], sdt, kind="ExternalInput")
    ws3 = nc.dram_tensor("ws3", [FS // WG, P, DT, WG], sdt, kind="ExternalInput")
    ws2 = nc.dram_tensor("ws2", [FS, D], sdt, kind="ExternalInput")
    bs1 = nc.dram_tensor("bs1", [P, FS // P], f32, kind="ExternalInput")
    bs3 = nc.dram_tensor("bs3", [P, FS // P], f32, kind="ExternalInput")
    bs2 = nc.dram_tensor("bs2", [P, D // P], f32, kind="ExternalInput")
    # spill quarter-job inputs
    xpT = nc.dram_tensor("xpT", [P, DT, CS], sdt, kind="ExternalInput")
    gws = nc.dram_tensor("gws", [1, CS], sdt, kind="ExternalInput")
    w1q = nc.dram_tensor("w1q", [P, DT, FH], sdt, kind="ExternalInput")
    w3q = nc.dram_tensor("w3q", [P, DT, FH], sdt, kind="ExternalInput")
    w2q = nc.dram_tensor("w2q", [P, FHT, D], sdt, kind="ExternalInput")
    b1q = nc.dram_tensor("b1q", [P, FHT], f32, kind="ExternalInput")
    b3q = nc.dram_tensor("b3q", [P, FHT], f32, kind="ExternalInput")
    b2q = nc.dram_tensor("b2q", [P, DT], f32, kind="ExternalInput")

    # ---- per-core outputs ----
    # bf16 outputs halve the store traffic; host combine upcasts to fp32.
    yT = nc.dram_tensor("yT", [D, CP], sdt, kind="ExternalOutput")
    ysT = nc.dram_tensor("ysT", [D, TS], sdt, kind="ExternalOutput")
    yqT = nc.dram_tensor("yqT", [D, CS], sdt, kind="ExternalOutput")

    CH = _chunks(CP)

    with tile.TileContext(nc) as tc:
        with (
            tc.tile_pool(name="tmp3", bufs=3) as htmp,
            tc.tile_pool(name="ps", bufs=4, space="PSUM") as psp,
        ):
            ytmp = htmp
            consts = tc.alloc_tile_pool(name="statics", bufs=1)
            # pools with bounded lifetimes, released imperatively so their
            # teardown overlaps later compute instead of serializing at the
            # kernel end:
            #  - xs + the ws1/ws3 stream die after the shared h-stage (their
            #    SBUF zone is reused by the spill-job weights)
            #  - ws2 stream + spill weights die before the expert path
            w2stream = tc.alloc_tile_pool(name="w2stream", bufs=4)
            earlyp = tc.alloc_tile_pool(name="early", bufs=1)
            wstream = tc.alloc_tile_pool(name="wstream", bufs=3)
            # ---------- expert-path prefetch plumbing ----------
            # issued in ~0.3-0.5MB chunks threaded through the shared-expert
            # loops on the scalar queue so the transfers never outrun the
            # latency-critical ws1/ws3/ws2 streams feeding the PE.
            b1_sb = consts.tile([P, FT], f32)
            b3_sb = consts.tile([P, FT], f32)
            b2_sb = consts.tile([P, DT], f32)
            gw_sb = consts.tile([P, CP], sdt, tag="gw")
            xg_sb = consts.tile([P, DT, CP], sdt, tag="xg")
            w1_sb = consts.tile([P, DT, F], sdt, tag="w1res")
            w3_sb = consts.tile([P, DT, F], sdt, tag="w3res")
            w2_sb = consts.tile([P, FT, D], sdt, tag="w2res")
            pf_chunks = []
            ws2_pre = []
            for fs in range(4):
                wb = w2stream.tile([P, D], sdt, tag="ws2b")
                pf_chunks.append(
                    (wb[:], ws2[:].rearrange("(o p) d -> p o d", p=P)[:, fs, :])
                )
                ws2_pre.append(wb)
            for d in range(DT):
                pf_chunks.append((xg_sb[:, d, :], xgT[:, d, :]))
            for d in range(DT):
                pf_chunks.append((w1_sb[:, d, :], w1[:, d, :]))
            for d in range(DT):
                pf_chunks.append((w3_sb[:, d, :], w3[:, d, :]))
            pf_chunks.append((b1_sb[:], b1[:]))
            pf_chunks.append((b3_sb[:], b3[:]))
            pf_chunks.append((b2_sb[:], b2[:]))
            pf_chunks.append((gw_sb[:], gw[:].to_broadcast([P, CP])))
            for fi in range(0, FT, 2):
                pf_chunks.append((w2_sb[:, fi : fi + 2, :], w2[:, fi : fi + 2, :]))
            pf_i = [0]

            def issue_prefetch(n=1):
                while n > 0 and pf_i[0] < len(pf_chunks):
                    dst, src = pf_chunks[pf_i[0]]
                    nc.scalar.dma_start(dst, src)
                    pf_i[0] += 1
                    n -= 1

            # a-buffer: holds a_shared [P, FST, 512] during the shared stage,
            # then two rotating [P, FT, 512] slabs for the expert chunks.
            ab = consts.tile([P, FST * 512], sdt, tag="abuf")
            a_shared = ab[:, : FST * 512].rearrange("p (f n) -> p f n", f=FST)

            def a_expert(n):
                off = (n % 2) * (FT * 512)
                return ab[:, off : off + FT * 512].rearrange("p (f n) -> p f n", f=FT)

            bs1_sb = consts.tile([P, FST], f32)
            bs3_sb = consts.tile([P, FST], f32)
            bs2_sb = consts.tile([P, DT], f32)


            # ---------- shared-expert inputs first (PE starts on these) ----------
            ws1_g0 = wstream.tile([P, DT, WG], sdt, tag="ws1g")
            ws3_g0 = wstream.tile([P, DT, WG], sdt, tag="ws3g")
            xs_sb = earlyp.tile([P, DT, TS], sdt, tag="xs")
            nc.sync.dma_start(ws1_g0[:, :, :P], ws1[0][:, :, :P])
            nc.gpsimd.dma_start(xs_sb[:, DT // 2 :, :], xsT[:, DT // 2 :, :])
            nc.scalar.dma_start(xs_sb[:, : DT // 2, :], xsT[:, : DT // 2, :])
            nc.sync.dma_start(ws1_g0[:, :, P:], ws1[0][:, :, P:])
            nc.gpsimd.dma_start(ws3_g0[:, :, :P], ws3[0][:, :, :P])
            nc.gpsimd.dma_start(ws3_g0[:, :, P:], ws3[0][:, :, P:])
            nc.scalar.dma_start(bs1_sb[:], bs1[:])
            nc.scalar.dma_start(bs3_sb[:], bs3[:])
            nc.scalar.dma_start(bs2_sb[:], bs2[:])

            # PE clock warmup: the HAM gate holds the PE at 1.2GHz until it
            # has been ~3.4us busy. Dummy matmuls (reading the zeroed head of
            # the a-buffer) cover the initial input-DMA wait so the real
            # matmuls start at 2.4GHz even when the DMA subsystem starts slow.
            nc.vector.memset(ab[:, :TS], 0)
            pwarm = psp.tile([P, TS], f32, tag="ph1", name="pwarm")
            N_WARM = 20
            for i in range(N_WARM):
                nc.tensor.matmul(
                    pwarm[:, : 2 * P],
                    lhsT=ab[:, :P],
                    rhs=ab[:, : 2 * P],
                    start=(i == 0),
                    stop=(i == N_WARM - 1),
                )

            # ---------- shared expert: h1s/h3s -> a_shared ----------
            for g in range(FST // 2):
                if g == 0:
                    ws1_g, ws3_g = ws1_g0, ws3_g0
                else:
                    ws1_g = wstream.tile([P, DT, WG], sdt, tag="ws1g")
                    nc.sync.dma_start(ws1_g[:], ws1[g])
                    ws3_g = wstream.tile([P, DT, WG], sdt, tag="ws3g")
                    nc.sync.dma_start(ws3_g[:], ws3[g])
                if g >= 2:
                    issue_prefetch()
                for sub in range(2):
                    fs = g * 2 + sub
                    scols = slice(sub * P, (sub + 1) * P)
                    ph1 = psp.tile([P, TS], f32, tag="ph1")
                    ph3 = psp.tile([P, TS], f32, tag="ph3")
                    for d in range(DT):
                        nc.tensor.matmul(
                            ph1[:],
                            lhsT=ws1_g[:, d, scols],
                            rhs=xs_sb[:, d, :],
                            start=(d == 0),
                            stop=(d == DT - 1),
                        )
                    for d in range(DT):
                        nc.tensor.matmul(
                            ph3[:],
                            lhsT=ws3_g[:, d, scols],
                            rhs=xs_sb[:, d, :],
                            start=(d == 0),
                            stop=(d == DT - 1),
                        )
                    h1 = htmp.tile([P, TS], f32, tag="h1")
                    nc.vector.tensor_scalar_add(h1[:], ph1[:], bs1_sb[:, fs : fs + 1])
                    prod = htmp.tile([P, TS], f32, tag="prod")
                    nc.vector.scalar_tensor_tensor(
                        prod[:],
                        in0=ph3[:],
                        scalar=bs3_sb[:, fs : fs + 1],
                        in1=h1[:],
                        op0=mybir.AluOpType.add,
                        op1=mybir.AluOpType.mult,
                    )
                    nc.scalar.activation(
                        a_shared[:, fs, :],
                        prod[:],
                        mybir.ActivationFunctionType.Silu,
                    )


            xp_sb = consts.tile([P, DT, CS], sdt, tag="xp")
            gws_sb = consts.tile([P, CS], sdt, tag="gws")
            b1q_sb = consts.tile([P, FHT], f32)
            b3q_sb = consts.tile([P, FHT], f32)
            b2q_sb = consts.tile([P, DT], f32)

            # ---------- shared expert: ys = a_shared @ Ws2 ----------
            # single pass over Ws2 (streamed once) accumulating all 8 d-tiles
            # in 8 PSUM banks at once
            pys = [
                psp.tile(
                    [P, TS], f32, tag=("ph1" if i < 4 else "ph3"), name=f"pys_{i}"
                )
                for i in range(DT)
            ]
            for fs in range(FST):
                if fs < len(ws2_pre):
                    ws2_b = ws2_pre[fs]
                else:
                    ws2_b = w2stream.tile([P, D], sdt, tag="ws2b")
                    nc.sync.dma_start(
                        ws2_b[:],
                        ws2[:].rearrange("(o p) d -> p o d", p=P)[:, fs, :],
                    )
                if fs == 2:
                    # Release xs + the ws1/ws3 stream HERE (not at the h/y
                    # boundary): the release is a cross-engine rendezvous,
                    # and by fs=2 the vector/scalar shared-h tail has
                    # drained, so the tensor queue arrives last and never
                    # stalls. The spill weights then reuse the freed zone.
                    wstream.release()
                    earlyp.release()
                    spillw = tc.alloc_tile_pool(name="spillw", bufs=1)
                    w1q_sb = spillw.tile([P, DT, FH], sdt, tag="w1q")
                    w3q_sb = spillw.tile([P, DT, FH], sdt, tag="w3q")
                    w2q_sb = spillw.tile([P, FHT, D], sdt, tag="w2q")
                    spill_chunks = []
                    for d in range(0, DT, 4):
                        spill_chunks.append(
                            (w1q_sb[:, d : d + 4, :], w1q[:, d : d + 4, :])
                        )
                    for d in range(0, DT, 4):
                        spill_chunks.append(
                            (w3q_sb[:, d : d + 4, :], w3q[:, d : d + 4, :])
                        )
                    spill_chunks.append((w2q_sb[:, 0:2, :], w2q[:, 0:2, :]))
                    spill_chunks.append((w2q_sb[:, 2:4, :], w2q[:, 2:4, :]))
                    spill_chunks.append((xp_sb[:], xpT[:]))
                    spill_chunks.append((gws_sb[:], gws[:].to_broadcast([P, CS])))
                    spill_chunks.append((b1q_sb[:], b1q[:]))
                    spill_chunks.append((b3q_sb[:], b3q[:]))
                    spill_chunks.append((b2q_sb[:], b2q[:]))
                    # ahead of the primary-w2 chunks: the spill stage runs
                    # right after shared-y, while primary w2 isn't read until
                    # ~60us into the expert path
                    ins_at = max(3 * DT + 8, pf_i[0])
                    pf_chunks[ins_at:ins_at] = spill_chunks
                for d in range(DT):
                    nc.tensor.matmul(
                        pys[d][:],
                        lhsT=ws2_b[:, d * P : (d + 1) * P],
                        rhs=a_shared[:, fs, :],
                        start=(fs == 0),
                        stop=(fs == FST - 1),
                    )
                # issued after the matmuls: the Tile tracker folds DMA-queue
                # semaphore waits conservatively, so chunks issued ahead of
                # the matmuls in program order would stall them on unrelated
                # transfers
                if fs >= 1:
                    issue_prefetch()
            for dp in range(DT // 2):
                dA = 2 * dp
                yo2 = ytmp.tile([P, 2, TS], sdt, tag="yo2")
                nc.vector.tensor_scalar_add(
                    yo2[:, 0, :], pys[dA][:], bs2_sb[:, dA : dA + 1]
                )
                nc.vector.tensor_scalar_add(
                    yo2[:, 1, :], pys[dA + 1][:], bs2_sb[:, dA + 1 : dA + 2]
                )
                nc.gpsimd.dma_start(
                    ysT[:].rearrange("(o p) c -> p o c", p=P)[:, dA : dA + 2, :],
                    yo2[:],
                )



            # ---------- spill quarter-job: deferred emission ----------
            # each step is emitted interleaved into the expert chunk-0
            # h-stage so the spill's vector/scalar chain (which dominates its
            # tiny 64-col matmuls) hides under the expert matmuls
            aq = consts.tile([P, FHT, CS], sdt, tag="aq")

            def spill_h_step(f):
                ph1 = psp.tile([P, TS], f32, tag="ph1")
                ph3 = psp.tile([P, TS], f32, tag="ph3")
                fcols = slice(f * P, (f + 1) * P)
                for d in range(DT):
                    nc.tensor.matmul(
                        ph1[:, :CS],
                        lhsT=w1q_sb[:, d, fcols],
                        rhs=xp_sb[:, d, :],
                        start=(d == 0),
                        stop=(d == DT - 1),
                    )
                for d in range(DT):
                    nc.tensor.matmul(
                        ph3[:, :CS],
                        lhsT=w3q_sb[:, d, fcols],
                        rhs=xp_sb[:, d, :],
                        start=(d == 0),
                        stop=(d == DT - 1),
                    )
                h1 = htmp.tile([P, TS], f32, tag="h1")
                nc.vector.tensor_scalar_add(
                    h1[:, :CS], ph1[:, :CS], b1q_sb[:, f : f + 1]
                )
                prod = htmp.tile([P, TS], f32, tag="prod")
                nc.vector.scalar_tensor_tensor(
                    prod[:, :CS],
                    in0=ph3[:, :CS],
                    scalar=b3q_sb[:, f : f + 1],
                    in1=h1[:, :CS],
                    op0=mybir.AluOpType.add,
                    op1=mybir.AluOpType.mult,
                )
                nc.scalar.activation(
                    aq[:, f, :], prod[:, :CS], mybir.ActivationFunctionType.Silu
                )

            def spill_y_step(dp):
                dA, dB = 2 * dp, 2 * dp + 1
                pyA = psp.tile([P, TS], f32, tag="ph1")
                pyB = psp.tile([P, TS], f32, tag="ph3")
                for f in range(FHT):
                    nc.tensor.matmul(
                        pyA[:, :CS],
                        lhsT=w2q_sb[:, f, dA * P : (dA + 1) * P],
                        rhs=aq[:, f, :],
                        start=(f == 0),
                        stop=(f == FHT - 1),
                    )
                    nc.tensor.matmul(
                        pyB[:, :CS],
                        lhsT=w2q_sb[:, f, dB * P : (dB + 1) * P],
                        rhs=aq[:, f, :],
                        start=(f == 0),
                        stop=(f == FHT - 1),
                    )
                yo2 = ytmp.tile([P, 2, TS], sdt, tag="yo2")
                for k, (d, py) in enumerate(((dA, pyA), (dB, pyB))):
                    nc.vector.scalar_tensor_tensor(
                        yo2[:, k, :CS],
                        in0=py[:, :CS],
                        scalar=b2q_sb[:, d : d + 1],
                        in1=gws_sb[:, :],
                        op0=mybir.AluOpType.add,
                        op1=mybir.AluOpType.mult,
                    )
                eng = nc.sync if dp % 2 == 0 else nc.gpsimd
                eng.dma_start(
                    yqT[:].rearrange("(o p) c -> p o c", p=P)[:, dA : dB + 1, :],
                    yo2[:, :, :CS],
                )

            spill_steps = [lambda f=f: spill_h_step(f) for f in range(FHT)]
            spill_steps += [lambda dp=dp: spill_y_step(dp) for dp in range(DT // 2)]

            # ---------- expert path ----------
            for n, (cs_, cw) in enumerate(CH):
                a_n = a_expert(n)
                ncols = slice(cs_, cs_ + cw)
                for f in range(FT):
                    ph1 = psp.tile([P, TS], f32, tag="ph1")
                    ph3 = psp.tile([P, TS], f32, tag="ph3")
                    fcols = slice(f * P, (f + 1) * P)
                    for d in range(DT):
                        nc.tensor.matmul(
                            ph1[:, :cw],
                            lhsT=w1_sb[:, d, fcols],
                            rhs=xg_sb[:, d, ncols],
                            start=(d == 0),
                            stop=(d == DT - 1),
                        )
                    for d in range(DT):
                        nc.tensor.matmul(
                            ph3[:, :cw],
                            lhsT=w3_sb[:, d, fcols],
                            rhs=xg_sb[:, d, ncols],
                            start=(d == 0),
                            stop=(d == DT - 1),
                        )
                    h1 = htmp.tile([P, TS], f32, tag="h1")
                    nc.vector.tensor_scalar_add(
                        h1[:, :cw], ph1[:, :cw], b1_sb[:, f : f + 1]
                    )
                    prod = htmp.tile([P, TS], f32, tag="prod")
                    nc.vector.scalar_tensor_tensor(
                        prod[:, :cw],
                        in0=ph3[:, :cw],
                        scalar=b3_sb[:, f : f + 1],
                        in1=h1[:, :cw],
                        op0=mybir.AluOpType.add,
                        op1=mybir.AluOpType.mult,
                    )
                    nc.scalar.activation(
                        a_n[:, f, :cw],
                        prod[:, :cw],
                        mybir.ActivationFunctionType.Silu,
                    )
                    if f % 2 == 1 and spill_steps:
                        spill_steps.pop(0)()
                    if n == 0:
                        issue_prefetch()
                if n == 0:
                    issue_prefetch(len(pf_chunks))
                    spillw.release()
                    w2stream.release()
                for dp in range(DT // 2):
                    dA, dB = 2 * dp, 2 * dp + 1
                    pyA = psp.tile([P, TS], f32, tag="ph1")
                    pyB = psp.tile([P, TS], f32, tag="ph3")
                    for f in range(FT):
                        nc.tensor.matmul(
                            pyA[:, :cw],
                            lhsT=w2_sb[:, f, dA * P : (dA + 1) * P],
                            rhs=a_n[:, f, :cw],
                            start=(f == 0),
                            stop=(f == FT - 1),
                        )
                        nc.tensor.matmul(
                            pyB[:, :cw],
                            lhsT=w2_sb[:, f, dB * P : (dB + 1) * P],
                            rhs=a_n[:, f, :cw],
                            start=(f == 0),
                            stop=(f == FT - 1),
                        )
                    yo2 = ytmp.tile([P, 2, TS], sdt, tag="yo2")
                    for k, (d, py) in enumerate(((dA, pyA), (dB, pyB))):
                        nc.vector.scalar_tensor_tensor(
                            yo2[:, k, :cw],
                            in0=py[:, :cw],
                            scalar=b2_sb[:, d : d + 1],
                            in1=gw_sb[:, ncols],
                            op0=mybir.AluOpType.add,
                            op1=mybir.AluOpType.mult,
                        )
                    eng = nc.sync if dp % 2 == 0 else nc.gpsimd
                    eng.dma_start(
                        yT[:].rearrange("(o p) c -> p o c", p=P)[
                            :, dA : dB + 1, ncols
                        ],
                        yo2[:, :, :cw],
                    )
            consts.release()

    nc.compile()
    return nc


def _get_program(CP, cfg):
    key = (CP, cfg)
    if key not in _COMPILED:
        _COMPILED[key] = build_program(CP, cfg)
    return _COMPILED[key]


def _pack_bias(b):
    """[K] -> [128, K/128] partition-major (element (p, o) = b[o*128+p])."""
    b = np.asarray(b, dtype=np.float32)
    return np.ascontiguousarray(b.reshape(-1, P).T)


def _route(xf, Wg):
    """Host gating: softmax -> top-2 -> renormalized weights (float64)."""
    logits = xf.astype(np.float64) @ Wg.astype(np.float64)
    m = logits.max(-1, keepdims=True)
    p = np.exp(logits - m)
    scores = p / p.sum(-1, keepdims=True)
    eidx = np.argsort(-scores, axis=-1, kind="stable")[:, :TOPK]
    sel = np.take_along_axis(scores, eidx, -1)
    sm = sel.max(-1, keepdims=True)
    pe = np.exp(sel - sm)
    ew = pe / pe.sum(-1, keepdims=True)
    return eidx, ew.astype(np.float32)


def prepare_in_maps(x, Wg, W1, b1, W3, b3, W2, b2, Ws1, bs1, Ws3, bs3, Ws2, bs2, cfg=MM_CFG):
    xf = np.ascontiguousarray(np.asarray(x, dtype=np.float32).reshape(-1, D))
    eidx, ew = _route(xf, np.asarray(Wg, dtype=np.float32))

    flat_e = eidx.reshape(-1)
    flat_w = ew.reshape(-1)
    tok = np.repeat(np.arange(T), TOPK)
    order = np.argsort(flat_e, kind="stable")
    se, st, sw = flat_e[order], tok[order], flat_w[order]
    counts = np.bincount(se, minlength=E)
    offs = np.concatenate([[0], np.cumsum(counts)])
    idx_lists = [st[offs[e] : offs[e + 1]] for e in range(E)]
    gw_lists = [sw[offs[e] : offs[e + 1]] for e in range(E)]

    CP, jobs = plan_spill(counts)

    np_mdt = _np_mm_dtype(cfg)

    def tile_kxn(a, K):
        # [K, N] -> [P, K/P, N] partition-major
        a = np.asarray(a, dtype=np.float32)
        return a.reshape(K // P, P, -1).transpose(1, 0, 2).astype(np_mdt)

    # shared-expert weights are identical on every core: pack once
    ws1_t = np.asarray(Ws1, dtype=np.float32).reshape(D // P, P, FS // WG, WG)
    ws1_t = ws1_t.transpose(2, 1, 0, 3).astype(np_mdt)
    ws3_t = np.asarray(Ws3, dtype=np.float32).reshape(D // P, P, FS // WG, WG)
    ws3_t = ws3_t.transpose(2, 1, 0, 3).astype(np_mdt)
    ws2_t = np.asarray(Ws2, dtype=np.float32).astype(np_mdt)
    bs1_p, bs3_p, bs2_p = _pack_bias(bs1), _pack_bias(bs3), _pack_bias(bs2)

    W1 = np.asarray(W1, dtype=np.float32)
    W3 = np.asarray(W3, dtype=np.float32)
    W2 = np.asarray(W2, dtype=np.float32)
    b1 = np.asarray(b1, dtype=np.float32)
    b3 = np.asarray(b3, dtype=np.float32)
    b2 = np.asarray(b2, dtype=np.float32)

    in_maps = []
    for c in range(E):
        cnt = min(int(counts[c]), CP)
        xg = np.zeros((CP, D), dtype=np.float32)
        xg[:cnt] = xf[idx_lists[c][:cnt]]
        gwv = np.zeros((1, CP), dtype=np.float32)
        gwv[0, :cnt] = gw_lists[c][:cnt]
        xsl = xf[c * TS : (c + 1) * TS]
        # spill quarter-job for this core
        xp = np.zeros((CS, D), dtype=np.float32)
        gqv = np.zeros((1, CS), dtype=np.float32)
        if c < len(jobs):
            e, q, s, w = jobs[c]
            xp[:w] = xf[idx_lists[e][s : s + w]]
            gqv[0, :w] = gw_lists[e][s : s + w]
            w1q_h = W1[e][:, q * FH : (q + 1) * FH]
            w3q_h = W3[e][:, q * FH : (q + 1) * FH]
            w2q_h = W2[e][q * FH : (q + 1) * FH, :]
            b1q_h = b1[e][q * FH : (q + 1) * FH]
            b3q_h = b3[e][q * FH : (q + 1) * FH]
            b2q_h = b2[e] if q == 0 else np.zeros_like(b2[e])
        else:
            w1q_h = np.zeros((D, FH), dtype=np.float32)
            w3q_h = np.zeros((D, FH), dtype=np.float32)
            w2q_h = np.zeros((FH, D), dtype=np.float32)
            b1q_h = np.zeros((FH,), dtype=np.float32)
            b3q_h = np.zeros((FH,), dtype=np.float32)
            b2q_h = np.zeros((D,), dtype=np.float32)
        in_maps.append(
            {
                "xgT": tile_kxn(xg.T, D),
                "gw": gwv.astype(np_mdt),
                "w1": tile_kxn(W1[c], D),
                "w3": tile_kxn(W3[c], D),
                "w2": tile_kxn(W2[c], F),
                "b1": _pack_bias(b1[c]),
                "b3": _pack_bias(b3[c]),
                "b2": _pack_bias(b2[c]),
                "xsT": tile_kxn(xsl.T, D),
                "ws1": ws1_t,
                "ws3": ws3_t,
                "ws2": ws2_t,
                "bs1": bs1_p,
                "bs3": bs3_p,
                "bs2": bs2_p,
                "xpT": tile_kxn(xp.T, D),
                "gws": gqv.astype(np_mdt),
                "w1q": tile_kxn(w1q_h, D),
                "w3q": tile_kxn(w3q_h, D),
                "w2q": tile_kxn(w2q_h, FH),
                "b1q": _pack_bias(b1q_h),
                "b3q": _pack_bias(b3q_h),
                "b2q": _pack_bias(b2q_h),
            }
        )
    return in_maps, idx_lists, counts, CP, jobs


def combine(results, idx_lists, counts, CP, jobs, x_shape, x_dtype):
    y = np.empty((D, T), dtype=np.float32)
    for c in range(E):
        y[:, c * TS : (c + 1) * TS] = np.asarray(results[c]["ysT"], dtype=np.float32)
    for c in range(E):
        cnt = min(int(counts[c]), CP)
        if cnt:
            cols = idx_lists[c][:cnt]
            y[:, cols] += np.asarray(results[c]["yT"][:, :cnt], dtype=np.float32)
    for c, job in enumerate(jobs):
        e, q, s, w = job
        cols = idx_lists[e][s : s + w]
        y[:, cols] += np.asarray(results[c]["yqT"][:, :w], dtype=np.float32)
    return np.ascontiguousarray(y.T).reshape(x_shape).astype(x_dtype, copy=False)


def run(x, Wg, W1, b1, W3, b3, W2, b2, Ws1, bs1, Ws3, bs3, Ws2, bs2,
        cfg=MM_CFG, trace=False, trace_kwargs=None):
    in_maps, idx_lists, counts, CP, jobs = prepare_in_maps(
        x, Wg, W1, b1, W3, b3, W2, b2, Ws1, bs1, Ws3, bs3, Ws2, bs2, cfg
    )
    nc = _get_program(CP, cfg)
    res = run_bass_kernel_spmd(
        nc, in_maps, list(range(E)), trace=trace, **(trace_kwargs or {})
    )
    out = combine(
        res.results, idx_lists, counts, CP, jobs,
        np.asarray(x).shape, np.asarray(x).dtype,
    )
    return out, res


def kernel(**inputs):
    out, _ = run(**inputs)
    return out


# revision 36
# speedup vs baseline: 1.0240x; 1.0240x over previous
"""MoE layer (top-2 of 8 experts + shared expert) as a Bass/Tile kernel on 8 TRN2 cores.

Strategy (expert parallelism, per the sharding hint):
  - Host computes the tiny gating network (softmax -> top-2 -> renormalize) and
    builds the all-to-all token dispatch: core e receives the tokens routed to
    expert e, pre-transposed to [d_model, C].
  - SPMD means every core executes the same instruction count, so the routed
    capacity is a compiled constant CP. Tokens beyond CP on overloaded experts
    ("spill") are computed as quarter-d_ff jobs spread over 4 cores each, so CP
    can sit at the mean expert load instead of the max (load balancing without
    duplicating full expert weights: a quarter weight set is 3MB).
  - Core e runs expert e's FFN on its tokens (silu(x@W1 * x@W3) @ W2), scales
    each output token by its gate weight, runs a 512-token slice of the shared
    expert (token-parallel across the 8 cores), and one spill quarter-job.
  - Host scatter-adds the expert/spill contributions per token and the shared
    output back into the full [T, d] result.

Device schedule: the shared expert runs FIRST (its weights stream in small
chunks, so the PE starts within ~2us), while the expert-path weights prefetch
in small chunks threaded through the stream queue slots; the expert path then
runs entirely from SBUF-resident weights. The spill job runs last (its tiny
epilogue minimizes the post-matmul drain tail).
"""

import os
import sys

for _p in ("/opt/trn_rl_repo",):
    if _p not in sys.path and os.path.isdir(_p):
        sys.path.insert(0, _p)

import numpy as np
import ml_dtypes

import concourse.bass as bass
import concourse.mybir as mybir
import concourse.tile as tile
from concourse import bacc
from concourse.bass_utils import run_bass_kernel_spmd


def install_ntff_hook():
    """This image's antenv lacks axon_hooks, which run_bass_kernel_spmd imports
    unconditionally when tracing; provide it and register the ctypes NTFF
    profile hook so trace=True (or BASS_TRACE=1) works."""
    import types

    try:
        import antenv.axon_hooks  # noqa: F401
        return
    except ImportError:
        pass
    mod = types.ModuleType("antenv.axon_hooks")
    _hook = [None]
    mod.set_axon_ntff_profile_hook = lambda h: _hook.__setitem__(0, h)
    mod.get_axon_ntff_profile_hook = lambda: _hook[0]
    sys.modules["antenv.axon_hooks"] = mod
    try:
        import antenv

        antenv.axon_hooks = mod
    except ImportError:
        pass
    try:
        from trn_agent_boot.trn_boot import _ntff_profile_via_ctypes

        mod.set_axon_ntff_profile_hook(
            _ntff_profile_via_ctypes("/opt/axon/libaxon_pjrt.so")
        )
    except Exception:
        pass


install_ntff_hook()

P = 128
D = 1024          # d_model
F = 2048          # d_ff per expert
FS = 4096         # shared expert hidden
E = 8             # experts == cores
TOPK = 2
TS = 512          # shared-expert tokens per core (T / 8)
T = 4096
CS = 64           # spill-job token capacity per core
NQ = 4            # spill splits d_ff into NQ quarter-jobs
FH = F // NQ      # 512: spill-job hidden width

DT = D // P       # 8
FT = F // P       # 16
FST = FS // P     # 32
FHT = FH // P     # 4
WG = 2 * P        # ws1/ws3 streaming group width

f32 = mybir.dt.float32
bf16 = mybir.dt.bfloat16

MM_CFG = os.environ.get("MOE_MM_CFG", "bf16")

_COMPILED: dict = {}


def _np_mm_dtype(cfg):
    return ml_dtypes.bfloat16 if cfg == "bf16" else np.float32


def _chunks(C):
    """Split C token columns into near-equal matmul-N chunks of <=512."""
    n = -(-C // 512)
    out = []
    s = 0
    for i in range(n):
        if i < n - 1:
            w = (-(-(C - s) // (n - i)) + 7) // 8 * 8
        else:
            w = C - s
        out.append((s, w))
        s += w
    return out


def plan_spill(counts):
    """Pick the compiled primary capacity CP and the spill quarter-jobs.

    Every token-expert pair beyond CP on an expert is computed as NQ
    quarter-d_ff jobs on NQ different cores (each core has one CS-column spill
    slot). Returns (CP, jobs) with jobs a list of (expert, quarter,
    token_start, width), len(jobs) <= E.
    """
    CP = max(8, int(-(-int(np.sum(counts)) // E // 8) * 8))
    while True:
        jobs = []
        for e in range(E):
            O = int(counts[e]) - CP
            s = CP
            while O > 0:
                w = min(CS, O)
                for q in range(NQ):
                    jobs.append((e, q, s, w))
                O -= w
                s += w
        if len(jobs) <= E:
            return CP, jobs
        CP += 8


def build_program(CP: int, cfg: str):
    """Build the per-core Bass program for primary capacity CP (+CS spill)."""
    assert cfg == "bf16"
    assert CP % 8 == 0
    sdt = bf16

    nc = bacc.Bacc("TRN2", target_bir_lowering=False, debug=False, num_devices=E)

    # ---- per-core inputs ----
    # All inputs arrive host-pretiled in the exact SBUF layout (partition
    # dim first) so every DMA is one contiguous per-partition segment.
    xgT = nc.dram_tensor("xgT", [P, DT, CP], sdt, kind="ExternalInput")
    gw = nc.dram_tensor("gw", [1, CP], sdt, kind="ExternalInput")
    w1 = nc.dram_tensor("w1", [P, DT, F], sdt, kind="ExternalInput")
    w3 = nc.dram_tensor("w3", [P, DT, F], sdt, kind="ExternalInput")
    w2 = nc.dram_tensor("w2", [P, FT, D], sdt, kind="ExternalInput")
    b1 = nc.dram_tensor("b1", [P, F // P], f32, kind="ExternalInput")
    b3 = nc.dram_tensor("b3", [P, F // P], f32, kind="ExternalInput")
    b2 = nc.dram_tensor("b2", [P, D // P], f32, kind="ExternalInput")
    xsT = nc.dram_tensor("xsT", [P, DT, TS], sdt, kind="ExternalInput")
    ws1 = nc.dram_tensor("ws1", [FS // WG, P, DT, WG], sdt, kind="ExternalInput")
    ws3 = nc.dram_tensor("ws3", [FS // WG, P, DT, WG], sdt, kind="ExternalInput")
    ws2 = nc.dram_tensor("ws2", [FS, D], sdt, kind="ExternalInput")
    bs1 = nc.dram_tensor("bs1", [P, FS // P], f32, kind="ExternalInput")
    bs3 = nc.dram_tensor("bs3", [P, FS // P], f32, kind="ExternalInput")
    bs2 = nc.dram_tensor("bs2", [P, D // P], f32, kind="ExternalInput")
    # spill quarter-job inputs
    xpT = nc.dram_tensor("xpT", [P, DT, CS], sdt, kind="ExternalInput")
    gws = nc.dram_tensor("gws", [1, CS], sdt, kind="ExternalInput")
    w1q = nc.dram_tensor("w1q", [P, DT, FH], sdt, kind="ExternalInput")
    w3q = nc.dram_tensor("w3q", [P, DT, FH], sdt, kind="ExternalInput")
    w2q = nc.dram_tensor("w2q", [P, FHT, D], sdt, kind="ExternalInput")
    b1q = nc.dram_tensor("b1q", [P, FHT], f32, kind="ExternalInput")
    b3q = nc.dram_tensor("b3q", [P, FHT], f32, kind="ExternalInput")
    b2q = nc.dram_tensor("b2q", [P, DT], f32, kind="ExternalInput")

    # ---- per-core outputs ----
    # bf16 outputs halve the store traffic; host combine upcasts to fp32.
    yT = nc.dram_tensor("yT", [D, CP], sdt, kind="ExternalOutput")
    ysT = nc.dram_tensor("ysT", [D, TS], sdt, kind="ExternalOutput")
    yqT = nc.dram_tensor("yqT", [D, CS], sdt, kind="ExternalOutput")

    CH = _chunks(CP)

    with tile.TileContext(nc) as tc:
        with (
            tc.tile_pool(name="tmp3", bufs=3) as htmp,
            tc.tile_pool(name="ps", bufs=4, space="PSUM") as psp,
        ):
            ytmp = htmp
            consts = tc.alloc_tile_pool(name="statics", bufs=1)
            # pools with bounded lifetimes, released imperatively so their
            # teardown overlaps later compute instead of serializing at the
            # kernel end:
            #  - xs + the ws1/ws3 stream die after the shared h-stage (their
            #    SBUF zone is reused by the spill-job weights)
            #  - ws2 stream + spill weights die before the expert path
            w2stream = tc.alloc_tile_pool(name="w2stream", bufs=4)
            earlyp = tc.alloc_tile_pool(name="early", bufs=1)
            wstream = tc.alloc_tile_pool(name="wstream", bufs=3)
            # ---------- expert-path prefetch plumbing ----------
            # issued in ~0.3-0.5MB chunks threaded through the shared-expert
            # loops on the scalar queue so the transfers never outrun the
            # latency-critical ws1/ws3/ws2 streams feeding the PE.
            b1_sb = consts.tile([P, FT], f32)
            b3_sb = consts.tile([P, FT], f32)
            b2_sb = consts.tile([P, DT], f32)
            gw_sb = consts.tile([P, CP], sdt, tag="gw")
            xg_sb = consts.tile([P, DT, CP], sdt, tag="xg")
            w1_sb = consts.tile([P, DT, F], sdt, tag="w1res")
            w3_sb = consts.tile([P, DT, F], sdt, tag="w3res")
            w2_sb = consts.tile([P, FT, D], sdt, tag="w2res")
            pf_chunks = []
            ws2_pre = []
            for fs in range(4):
                wb = w2stream.tile([P, D], sdt, tag="ws2b")
                pf_chunks.append(
                    (wb[:], ws2[:].rearrange("(o p) d -> p o d", p=P)[:, fs, :])
                )
                ws2_pre.append(wb)
            for d in range(DT):
                pf_chunks.append((xg_sb[:, d, :], xgT[:, d, :]))
            for d in range(DT):
                pf_chunks.append((w1_sb[:, d, :], w1[:, d, :]))
            for d in range(DT):
                pf_chunks.append((w3_sb[:, d, :], w3[:, d, :]))
            pf_chunks.append((b1_sb[:], b1[:]))
            pf_chunks.append((b3_sb[:], b3[:]))
            pf_chunks.append((b2_sb[:], b2[:]))
            pf_chunks.append((gw_sb[:], gw[:].to_broadcast([P, CP])))
            for fi in range(0, FT, 2):
                pf_chunks.append((w2_sb[:, fi : fi + 2, :], w2[:, fi : fi + 2, :]))
            pf_i = [0]

            def issue_prefetch(n=1):
                while n > 0 and pf_i[0] < len(pf_chunks):
                    dst, src = pf_chunks[pf_i[0]]
                    nc.scalar.dma_start(dst, src)
                    pf_i[0] += 1
                    n -= 1

            # a-buffer: holds a_shared [P, FST, 512] during the shared stage,
            # then two rotating [P, FT, 512] slabs for the expert chunks.
            ab = consts.tile([P, FST * 512], sdt, tag="abuf")
            a_shared = ab[:, : FST * 512].rearrange("p (f n) -> p f n", f=FST)

            def a_expert(n):
                off = (n % 2) * (FT * 512)
                return ab[:, off : off + FT * 512].rearrange("p (f n) -> p f n", f=FT)

            bs1_sb = consts.tile([P, FST], f32)
            bs3_sb = consts.tile([P, FST], f32)
            bs2_sb = consts.tile([P, DT], f32)


            # ---------- shared-expert inputs first (PE starts on these) ----------
            ws1_g0 = wstream.tile([P, DT, WG], sdt, tag="ws1g")
            ws3_g0 = wstream.tile([P, DT, WG], sdt, tag="ws3g")
            xs_sb = earlyp.tile([P, DT, TS], sdt, tag="xs")
            nc.sync.dma_start(ws1_g0[:, :, :P], ws1[0][:, :, :P])
            nc.gpsimd.dma_start(xs_sb[:, DT // 2 :, :], xsT[:, DT // 2 :, :])
            nc.scalar.dma_start(xs_sb[:, : DT // 2, :], xsT[:, : DT // 2, :])
            nc.sync.dma_start(ws1_g0[:, :, P:], ws1[0][:, :, P:])
            nc.gpsimd.dma_start(ws3_g0[:, :, :P], ws3[0][:, :, :P])
            nc.gpsimd.dma_start(ws3_g0[:, :, P:], ws3[0][:, :, P:])
            nc.scalar.dma_start(bs1_sb[:], bs1[:])
            nc.scalar.dma_start(bs3_sb[:], bs3[:])
            nc.scalar.dma_start(bs2_sb[:], bs2[:])

            # PE clock warmup: the HAM gate holds the PE at 1.2GHz until it
            # has been ~3.4us busy. Dummy matmuls (reading the zeroed head of
            # the a-buffer) cover the initial input-DMA wait so the real
            # matmuls start at 2.4GHz even when the DMA subsystem starts slow.
            nc.vector.memset(ab[:, :TS], 0)
            pwarm = psp.tile([P, TS], f32, tag="ph1", name="pwarm")
            N_WARM = 20
            for i in range(N_WARM):
                nc.tensor.matmul(
                    pwarm[:, : 2 * P],
                    lhsT=ab[:, :P],
                    rhs=ab[:, : 2 * P],
                    start=(i == 0),
                    stop=(i == N_WARM - 1),
                )

            # ---------- shared expert: h1s/h3s -> a_shared ----------
            for g in range(FST // 2):
                if g == 0:
                    ws1_g, ws3_g = ws1_g0, ws3_g0
                else:
                    ws1_g = wstream.tile([P, DT, WG], sdt, tag="ws1g")
                    nc.sync.dma_start(ws1_g[:], ws1[g])
                    ws3_g = wstream.tile([P, DT, WG], sdt, tag="ws3g")
                    nc.sync.dma_start(ws3_g[:], ws3[g])
                if g >= 2:
                    issue_prefetch()
                for sub in range(2):
                    fs = g * 2 + sub
                    scols = slice(sub * P, (sub + 1) * P)
                    ph1 = psp.tile([P, TS], f32, tag="ph1")
                    ph3 = psp.tile([P, TS], f32, tag="ph3")
                    for d in range(DT):
                        nc.tensor.matmul(
                            ph1[:],
                            lhsT=ws1_g[:, d, scols],
                            rhs=xs_sb[:, d, :],
                            start=(d == 0),
                            stop=(d == DT - 1),
                        )
                    for d in range(DT):
                        nc.tensor.matmul(
                            ph3[:],
                            lhsT=ws3_g[:, d, scols],
                            rhs=xs_sb[:, d, :],
                            start=(d == 0),
                            stop=(d == DT - 1),
                        )
                    h1 = htmp.tile([P, TS], f32, tag="h1")
                    nc.vector.tensor_scalar_add(h1[:], ph1[:], bs1_sb[:, fs : fs + 1])
                    prod = htmp.tile([P, TS], f32, tag="prod")
                    nc.vector.scalar_tensor_tensor(
                        prod[:],
                        in0=ph3[:],
                        scalar=bs3_sb[:, fs : fs + 1],
                        in1=h1[:],
                        op0=mybir.AluOpType.add,
                        op1=mybir.AluOpType.mult,
                    )
                    nc.scalar.activation(
                        a_shared[:, fs, :],
                        prod[:],
                        mybir.ActivationFunctionType.Silu,
                    )


            xp_sb = consts.tile([P, DT, CS], sdt, tag="xp")
            gws_sb = consts.tile([P, CS], sdt, tag="gws")
            b1q_sb = consts.tile([P, FHT], f32)
            b3q_sb = consts.tile([P, FHT], f32)
            b2q_sb = consts.tile([P, DT], f32)

            # ---------- shared expert: ys = a_shared @ Ws2 ----------
            # single pass over Ws2 (streamed once) accumulating all 8 d-tiles
            # in 8 PSUM banks at once
            pys = [
                psp.tile(
                    [P, TS], f32, tag=("ph1" if i < 4 else "ph3"), name=f"pys_{i}"
                )
                for i in range(DT)
            ]
            for fs in range(FST):
                if fs < len(ws2_pre):
                    ws2_b = ws2_pre[fs]
                else:
                    ws2_b = w2stream.tile([P, D], sdt, tag="ws2b")
                    nc.sync.dma_start(
                        ws2_b[:],
                        ws2[:].rearrange("(o p) d -> p o d", p=P)[:, fs, :],
                    )
                if fs == 2:
                    # Release xs + the ws1/ws3 stream HERE (not at the h/y
                    # boundary): the release is a cross-engine rendezvous,
                    # and by fs=2 the vector/scalar shared-h tail has
                    # drained, so the tensor queue arrives last and never
                    # stalls. The spill weights then reuse the freed zone.
                    wstream.release()
                    earlyp.release()
                    spillw = tc.alloc_tile_pool(name="spillw", bufs=1)
                    w1q_sb = spillw.tile([P, DT, FH], sdt, tag="w1q")
                    w3q_sb = spillw.tile([P, DT, FH], sdt, tag="w3q")
                    w2q_sb = spillw.tile([P, FHT, D], sdt, tag="w2q")
                    spill_chunks = []
                    for d in range(0, DT, 4):
                        spill_chunks.append(
                            (w1q_sb[:, d : d + 4, :], w1q[:, d : d + 4, :])
                        )
                    for d in range(0, DT, 4):
                        spill_chunks.append(
                            (w3q_sb[:, d : d + 4, :], w3q[:, d : d + 4, :])
                        )
                    spill_chunks.append((w2q_sb[:, 0:2, :], w2q[:, 0:2, :]))
                    spill_chunks.append((w2q_sb[:, 2:4, :], w2q[:, 2:4, :]))
                    spill_chunks.append((xp_sb[:], xpT[:]))
                    spill_chunks.append((gws_sb[:], gws[:].to_broadcast([P, CS])))
                    spill_chunks.append((b1q_sb[:], b1q[:]))
                    spill_chunks.append((b3q_sb[:], b3q[:]))
                    spill_chunks.append((b2q_sb[:], b2q[:]))
                    # ahead of the primary-w2 chunks: the spill stage runs
                    # right after shared-y, while primary w2 isn't read until
                    # ~60us into the expert path
                    ins_at = max(3 * DT + 8, pf_i[0])
                    pf_chunks[ins_at:ins_at] = spill_chunks
                issue_prefetch()
                for d in range(DT):
                    nc.tensor.matmul(
                        pys[d][:],
                        lhsT=ws2_b[:, d * P : (d + 1) * P],
                        rhs=a_shared[:, fs, :],
                        start=(fs == 0),
                        stop=(fs == FST - 1),
                    )
            for dp in range(DT // 2):
                dA = 2 * dp
                yo2 = ytmp.tile([P, 2, TS], sdt, tag="yo2")
                nc.vector.tensor_scalar_add(
                    yo2[:, 0, :], pys[dA][:], bs2_sb[:, dA : dA + 1]
                )
                nc.vector.tensor_scalar_add(
                    yo2[:, 1, :], pys[dA + 1][:], bs2_sb[:, dA + 1 : dA + 2]
                )
                nc.gpsimd.dma_start(
                    ysT[:].rearrange("(o p) c -> p o c", p=P)[:, dA : dA + 2, :],
                    yo2[:],
                )



            # flush any prefetch chunks the shared loops didn't cover
            issue_prefetch(len(pf_chunks))

            # ---------- spill quarter-job: deferred emission ----------
            # each step is emitted interleaved into the expert chunk-0
            # h-stage so the spill's vector/scalar chain (which dominates its
            # tiny 64-col matmuls) hides under the expert matmuls
            aq = consts.tile([P, FHT, CS], sdt, tag="aq")

            def spill_h_step(f):
                ph1 = psp.tile([P, TS], f32, tag="ph1")
                ph3 = psp.tile([P, TS], f32, tag="ph3")
                fcols = slice(f * P, (f + 1) * P)
                for d in range(DT):
                    nc.tensor.matmul(
                        ph1[:, :CS],
                        lhsT=w1q_sb[:, d, fcols],
                        rhs=xp_sb[:, d, :],
                        start=(d == 0),
                        stop=(d == DT - 1),
                    )
                for d in range(DT):
                    nc.tensor.matmul(
                        ph3[:, :CS],
                        lhsT=w3q_sb[:, d, fcols],
                        rhs=xp_sb[:, d, :],
                        start=(d == 0),
                        stop=(d == DT - 1),
                    )
                h1 = htmp.tile([P, TS], f32, tag="h1")
                nc.vector.tensor_scalar_add(
                    h1[:, :CS], ph1[:, :CS], b1q_sb[:, f : f + 1]
                )
                prod = htmp.tile([P, TS], f32, tag="prod")
                nc.vector.scalar_tensor_tensor(
                    prod[:, :CS],
                    in0=ph3[:, :CS],
                    scalar=b3q_sb[:, f : f + 1],
                    in1=h1[:, :CS],
                    op0=mybir.AluOpType.add,
                    op1=mybir.AluOpType.mult,
                )
                nc.scalar.activation(
                    aq[:, f, :], prod[:, :CS], mybir.ActivationFunctionType.Silu
                )

            def spill_y_step(dp):
                dA, dB = 2 * dp, 2 * dp + 1
                pyA = psp.tile([P, TS], f32, tag="ph1")
                pyB = psp.tile([P, TS], f32, tag="ph3")
                for f in range(FHT):
                    nc.tensor.matmul(
                        pyA[:, :CS],
                        lhsT=w2q_sb[:, f, dA * P : (dA + 1) * P],
                        rhs=aq[:, f, :],
                        start=(f == 0),
                        stop=(f == FHT - 1),
                    )
                    nc.tensor.matmul(
                        pyB[:, :CS],
                        lhsT=w2q_sb[:, f, dB * P : (dB + 1) * P],
                        rhs=aq[:, f, :],
                        start=(f == 0),
                        stop=(f == FHT - 1),
                    )
                yo2 = ytmp.tile([P, 2, TS], sdt, tag="yo2")
                for k, (d, py) in enumerate(((dA, pyA), (dB, pyB))):
                    nc.vector.scalar_tensor_tensor(
                        yo2[:, k, :CS],
                        in0=py[:, :CS],
                        scalar=b2q_sb[:, d : d + 1],
                        in1=gws_sb[:, :],
                        op0=mybir.AluOpType.add,
                        op1=mybir.AluOpType.mult,
                    )
                eng = nc.sync if dp % 2 == 0 else nc.gpsimd
                eng.dma_start(
                    yqT[:].rearrange("(o p) c -> p o c", p=P)[:, dA : dB + 1, :],
                    yo2[:, :, :CS],
                )

            spill_steps = [lambda f=f: spill_h_step(f) for f in range(FHT)]
            spill_steps += [lambda dp=dp: spill_y_step(dp) for dp in range(DT // 2)]

            # ---------- expert path ----------
            for n, (cs_, cw) in enumerate(CH):
                a_n = a_expert(n)
                ncols = slice(cs_, cs_ + cw)
                for f in range(FT):
                    ph1 = psp.tile([P, TS], f32, tag="ph1")
                    ph3 = psp.tile([P, TS], f32, tag="ph3")
                    fcols = slice(f * P, (f + 1) * P)
                    for d in range(DT):
                        nc.tensor.matmul(
                            ph1[:, :cw],
                            lhsT=w1_sb[:, d, fcols],
                            rhs=xg_sb[:, d, ncols],
                            start=(d == 0),
                            stop=(d == DT - 1),
                        )
                    for d in range(DT):
                        nc.tensor.matmul(
                            ph3[:, :cw],
                            lhsT=w3_sb[:, d, fcols],
                            rhs=xg_sb[:, d, ncols],
                            start=(d == 0),
                            stop=(d == DT - 1),
                        )
                    h1 = htmp.tile([P, TS], f32, tag="h1")
                    nc.vector.tensor_scalar_add(
                        h1[:, :cw], ph1[:, :cw], b1_sb[:, f : f + 1]
                    )
                    prod = htmp.tile([P, TS], f32, tag="prod")
                    nc.vector.scalar_tensor_tensor(
                        prod[:, :cw],
                        in0=ph3[:, :cw],
                        scalar=b3_sb[:, f : f + 1],
                        in1=h1[:, :cw],
                        op0=mybir.AluOpType.add,
                        op1=mybir.AluOpType.mult,
                    )
                    nc.scalar.activation(
                        a_n[:, f, :cw],
                        prod[:, :cw],
                        mybir.ActivationFunctionType.Silu,
                    )
                    if f % 2 == 1 and spill_steps:
                        spill_steps.pop(0)()
                    if n == 0:
                        issue_prefetch()
                if n == 0:
                    spillw.release()
                    w2stream.release()
                for dp in range(DT // 2):
                    dA, dB = 2 * dp, 2 * dp + 1
                    pyA = psp.tile([P, TS], f32, tag="ph1")
                    pyB = psp.tile([P, TS], f32, tag="ph3")
                    for f in range(FT):
                        nc.tensor.matmul(
                            pyA[:, :cw],
                            lhsT=w2_sb[:, f, dA * P : (dA + 1) * P],
                            rhs=a_n[:, f, :cw],
                            start=(f == 0),
                            stop=(f == FT - 1),
                        )
                        nc.tensor.matmul(
                            pyB[:, :cw],
                            lhsT=w2_sb[:, f, dB * P : (dB + 1) * P],
                            rhs=a_n[:, f, :cw],
                            start=(f == 0),
                            stop=(f == FT - 1),
                        )
                    yo2 = ytmp.tile([P, 2, TS], sdt, tag="yo2")
                    for k, (d, py) in enumerate(((dA, pyA), (dB, pyB))):
                        nc.vector.scalar_tensor_tensor(
                            yo2[:, k, :cw],
                            in0=py[:, :cw],
                            scalar=b2_sb[:, d : d + 1],
                            in1=gw_sb[:, ncols],
                            op0=mybir.AluOpType.add,
                            op1=mybir.AluOpType.mult,
                        )
                    eng = nc.sync if dp % 2 == 0 else nc.gpsimd
                    eng.dma_start(
                        yT[:].rearrange("(o p) c -> p o c", p=P)[
                            :, dA : dB + 1, ncols
                        ],
                        yo2[:, :, :cw],
                    )
            consts.release()

    nc.compile()
    return nc


def _get_program(CP, cfg):
    key = (CP, cfg)
    if key not in _COMPILED:
        _COMPILED[key] = build_program(CP, cfg)
    return _COMPILED[key]


def _pack_bias(b):
    """[K] -> [128, K/128] partition-major (element (p, o) = b[o*128+p])."""
    b = np.asarray(b, dtype=np.float32)
    return np.ascontiguousarray(b.reshape(-1, P).T)


def _route(xf, Wg):
    """Host gating: softmax -> top-2 -> renormalized weights (float64)."""
    logits = xf.astype(np.float64) @ Wg.astype(np.float64)
    m = logits.max(-1, keepdims=True)
    p = np.exp(logits - m)
    scores = p / p.sum(-1, keepdims=True)
    eidx = np.argsort(-scores, axis=-1, kind="stable")[:, :TOPK]
    sel = np.take_along_axis(scores, eidx, -1)
    sm = sel.max(-1, keepdims=True)
    pe = np.exp(sel - sm)
    ew = pe / pe.sum(-1, keepdims=True)
    return eidx, ew.astype(np.float32)


def prepare_in_maps(x, Wg, W1, b1, W3, b3, W2, b2, Ws1, bs1, Ws3, bs3, Ws2, bs2, cfg=MM_CFG):
    xf = np.ascontiguousarray(np.asarray(x, dtype=np.float32).reshape(-1, D))
    eidx, ew = _route(xf, np.asarray(Wg, dtype=np.float32))

    flat_e = eidx.reshape(-1)
    flat_w = ew.reshape(-1)
    tok = np.repeat(np.arange(T), TOPK)
    order = np.argsort(flat_e, kind="stable")
    se, st, sw = flat_e[order], tok[order], flat_w[order]
    counts = np.bincount(se, minlength=E)
    offs = np.concatenate([[0], np.cumsum(counts)])
    idx_lists = [st[offs[e] : offs[e + 1]] for e in range(E)]
    gw_lists = [sw[offs[e] : offs[e + 1]] for e in range(E)]

    CP, jobs = plan_spill(counts)

    np_mdt = _np_mm_dtype(cfg)

    def tile_kxn(a, K):
        # [K, N] -> [P, K/P, N] partition-major
        a = np.asarray(a, dtype=np.float32)
        return a.reshape(K // P, P, -1).transpose(1, 0, 2).astype(np_mdt)

    # shared-expert weights are identical on every core: pack once
    ws1_t = np.asarray(Ws1, dtype=np.float32).reshape(D // P, P, FS // WG, WG)
    ws1_t = ws1_t.transpose(2, 1, 0, 3).astype(np_mdt)
    ws3_t = np.asarray(Ws3, dtype=np.float32).reshape(D // P, P, FS // WG, WG)
    ws3_t = ws3_t.transpose(2, 1, 0, 3).astype(np_mdt)
    ws2_t = np.asarray(Ws2, dtype=np.float32).astype(np_mdt)
    bs1_p, bs3_p, bs2_p = _pack_bias(bs1), _pack_bias(bs3), _pack_bias(bs2)

    W1 = np.asarray(W1, dtype=np.float32)
    W3 = np.asarray(W3, dtype=np.float32)
    W2 = np.asarray(W2, dtype=np.float32)
    b1 = np.asarray(b1, dtype=np.float32)
    b3 = np.asarray(b3, dtype=np.float32)
    b2 = np.asarray(b2, dtype=np.float32)

    in_maps = []
    for c in range(E):
        cnt = min(int(counts[c]), CP)
        xg = np.zeros((CP, D), dtype=np.float32)
        xg[:cnt] = xf[idx_lists[c][:cnt]]
        gwv = np.zeros((1, CP), dtype=np.float32)
        gwv[0, :cnt] = gw_lists[c][:cnt]
        xsl = xf[c * TS : (c + 1) * TS]
        # spill quarter-job for this core
        xp = np.zeros((CS, D), dtype=np.float32)
        gqv = np.zeros((1, CS), dtype=np.float32)
        if c < len(jobs):
            e, q, s, w = jobs[c]
            xp[:w] = xf[idx_lists[e][s : s + w]]
            gqv[0, :w] = gw_lists[e][s : s + w]
            w1q_h = W1[e][:, q * FH : (q + 1) * FH]
            w3q_h = W3[e][:, q * FH : (q + 1) * FH]
            w2q_h = W2[e][q * FH : (q + 1) * FH, :]
            b1q_h = b1[e][q * FH : (q + 1) * FH]
            b3q_h = b3[e][q * FH : (q + 1) * FH]
            b2q_h = b2[e] if q == 0 else np.zeros_like(b2[e])
        else:
            w1q_h = np.zeros((D, FH), dtype=np.float32)
            w3q_h = np.zeros((D, FH), dtype=np.float32)
            w2q_h = np.zeros((FH, D), dtype=np.float32)
            b1q_h = np.zeros((FH,), dtype=np.float32)
            b3q_h = np.zeros((FH,), dtype=np.float32)
            b2q_h = np.zeros((D,), dtype=np.float32)
        in_maps.append(
            {
                "xgT": tile_kxn(xg.T, D),
                "gw": gwv.astype(np_mdt),
                "w1": tile_kxn(W1[c], D),
                "w3": tile_kxn(W3[c], D),
                "w2": tile_kxn(W2[c], F),
                "b1": _pack_bias(b1[c]),
                "b3": _pack_bias(b3[c]),
                "b2": _pack_bias(b2[c]),
                "xsT": tile_kxn(xsl.T, D),
                "ws1": ws1_t,
                "ws3": ws3_t,
                "ws2": ws2_t,
                "bs1": bs1_p,
                "bs3": bs3_p,
                "bs2": bs2_p,
                "xpT": tile_kxn(xp.T, D),
                "gws": gqv.astype(np_mdt),
                "w1q": tile_kxn(w1q_h, D),
                "w3q": tile_kxn(w3q_h, D),
                "w2q": tile_kxn(w2q_h, FH),
                "b1q": _pack_bias(b1q_h),
                "b3q": _pack_bias(b3q_h),
                "b2q": _pack_bias(b2q_h),
            }
        )
    return in_maps, idx_lists, counts, CP, jobs


def combine(results, idx_lists, counts, CP, jobs, x_shape, x_dtype):
    y = np.empty((D, T), dtype=np.float32)
    for c in range(E):
        y[:, c * TS : (c + 1) * TS] = np.asarray(results[c]["ysT"], dtype=np.float32)
    for c in range(E):
        cnt = min(int(counts[c]), CP)
        if cnt:
            cols = idx_lists[c][:cnt]
            y[:, cols] += np.asarray(results[c]["yT"][:, :cnt], dtype=np.float32)
    for c, job in enumerate(jobs):
        e, q, s, w = job
        cols = idx_lists[e][s : s + w]
        y[:, cols] += np.asarray(results[c]["yqT"][:, :w], dtype=np.float32)
    return np.ascontiguousarray(y.T).reshape(x_shape).astype(x_dtype, copy=False)


def run(x, Wg, W1, b1, W3, b3, W2, b2, Ws1, bs1, Ws3, bs3, Ws2, bs2,
        cfg=MM_CFG, trace=False, trace_kwargs=None):
    in_maps, idx_lists, counts, CP, jobs = prepare_in_maps(
        x, Wg, W1, b1, W3, b3, W2, b2, Ws1, bs1, Ws3, bs3, Ws2, bs2, cfg
    )
    nc = _get_program(CP, cfg)
    res = run_bass_kernel_spmd(
        nc, in_maps, list(range(E)), trace=trace, **(trace_kwargs or {})
    )
    out = combine(
        res.results, idx_lists, counts, CP, jobs,
        np.asarray(x).shape, np.asarray(x).dtype,
    )
    return out, res


def kernel(**inputs):
    out, _ = run(**inputs)
    return out


# revision 37
# speedup vs baseline: 1.0274x; 1.0033x over previous
"""MoE layer (top-2 of 8 experts + shared expert) as a Bass/Tile kernel on 8 TRN2 cores.

Strategy (expert parallelism, per the sharding hint):
  - Host computes the tiny gating network (softmax -> top-2 -> renormalize) and
    builds the all-to-all token dispatch: core e receives the tokens routed to
    expert e, pre-transposed to [d_model, C].
  - SPMD means every core executes the same instruction count, so the routed
    capacity is a compiled constant CP. Tokens beyond CP on overloaded experts
    ("spill") are computed as quarter-d_ff jobs spread over 4 cores each, so CP
    can sit at the mean expert load instead of the max (load balancing without
    duplicating full expert weights: a quarter weight set is 3MB).
  - Core e runs expert e's FFN on its tokens (silu(x@W1 * x@W3) @ W2), scales
    each output token by its gate weight, runs a 512-token slice of the shared
    expert (token-parallel across the 8 cores), and one spill quarter-job.
  - Host scatter-adds the expert/spill contributions per token and the shared
    output back into the full [T, d] result.

Device schedule: the shared expert runs FIRST (its weights stream in small
chunks, so the PE starts within ~2us), while the expert-path weights prefetch
in small chunks threaded through the stream queue slots; the expert path then
runs entirely from SBUF-resident weights. The spill job runs last (its tiny
epilogue minimizes the post-matmul drain tail).
"""

import os
import sys

for _p in ("/opt/trn_rl_repo",):
    if _p not in sys.path and os.path.isdir(_p):
        sys.path.insert(0, _p)

import numpy as np
import ml_dtypes

import concourse.bass as bass
import concourse.mybir as mybir
import concourse.tile as tile
from concourse import bacc
from concourse.bass_utils import run_bass_kernel_spmd


def install_ntff_hook():
    """This image's antenv lacks axon_hooks, which run_bass_kernel_spmd imports
    unconditionally when tracing; provide it and register the ctypes NTFF
    profile hook so trace=True (or BASS_TRACE=1) works."""
    import types

    try:
        import antenv.axon_hooks  # noqa: F401
        return
    except ImportError:
        pass
    mod = types.ModuleType("antenv.axon_hooks")
    _hook = [None]
    mod.set_axon_ntff_profile_hook = lambda h: _hook.__setitem__(0, h)
    mod.get_axon_ntff_profile_hook = lambda: _hook[0]
    sys.modules["antenv.axon_hooks"] = mod
    try:
        import antenv

        antenv.axon_hooks = mod
    except ImportError:
        pass
    try:
        from trn_agent_boot.trn_boot import _ntff_profile_via_ctypes

        mod.set_axon_ntff_profile_hook(
            _ntff_profile_via_ctypes("/opt/axon/libaxon_pjrt.so")
        )
    except Exception:
        pass


install_ntff_hook()

P = 128
D = 1024          # d_model
F = 2048          # d_ff per expert
FS = 4096         # shared expert hidden
E = 8             # experts == cores
TOPK = 2
TS = 512          # shared-expert tokens per core (T / 8)
T = 4096
CS = 64           # spill-job token capacity per core
NQ = 4            # spill splits d_ff into NQ quarter-jobs
FH = F // NQ      # 512: spill-job hidden width

DT = D // P       # 8
FT = F // P       # 16
FST = FS // P     # 32
FHT = FH // P     # 4
WG = 2 * P        # ws1/ws3 streaming group width

f32 = mybir.dt.float32
bf16 = mybir.dt.bfloat16

MM_CFG = os.environ.get("MOE_MM_CFG", "bf16")

_COMPILED: dict = {}


def _np_mm_dtype(cfg):
    return ml_dtypes.bfloat16 if cfg == "bf16" else np.float32


def _chunks(C):
    """Split C token columns into near-equal matmul-N chunks of <=512."""
    n = -(-C // 512)
    out = []
    s = 0
    for i in range(n):
        if i < n - 1:
            w = (-(-(C - s) // (n - i)) + 7) // 8 * 8
        else:
            w = C - s
        out.append((s, w))
        s += w
    return out


def plan_spill(counts):
    """Pick the compiled primary capacity CP and the spill quarter-jobs.

    Every token-expert pair beyond CP on an expert is computed as NQ
    quarter-d_ff jobs on NQ different cores (each core has one CS-column spill
    slot). Returns (CP, jobs) with jobs a list of (expert, quarter,
    token_start, width), len(jobs) <= E.
    """
    CP = max(8, int(-(-int(np.sum(counts)) // E // 8) * 8))
    while True:
        jobs = []
        for e in range(E):
            O = int(counts[e]) - CP
            s = CP
            while O > 0:
                w = min(CS, O)
                for q in range(NQ):
                    jobs.append((e, q, s, w))
                O -= w
                s += w
        if len(jobs) <= E:
            return CP, jobs
        CP += 8


def build_program(CP: int, cfg: str):
    """Build the per-core Bass program for primary capacity CP (+CS spill)."""
    assert cfg == "bf16"
    assert CP % 8 == 0
    sdt = bf16

    nc = bacc.Bacc("TRN2", target_bir_lowering=False, debug=False, num_devices=E)

    # ---- per-core inputs ----
    # All inputs arrive host-pretiled in the exact SBUF layout (partition
    # dim first) so every DMA is one contiguous per-partition segment.
    xgT = nc.dram_tensor("xgT", [P, DT, CP], sdt, kind="ExternalInput")
    gw = nc.dram_tensor("gw", [1, CP], sdt, kind="ExternalInput")
    w1 = nc.dram_tensor("w1", [P, DT, F], sdt, kind="ExternalInput")
    w3 = nc.dram_tensor("w3", [P, DT, F], sdt, kind="ExternalInput")
    w2 = nc.dram_tensor("w2", [P, FT, D], sdt, kind="ExternalInput")
    b1 = nc.dram_tensor("b1", [P, F // P], f32, kind="ExternalInput")
    b3 = nc.dram_tensor("b3", [P, F // P], f32, kind="ExternalInput")
    b2 = nc.dram_tensor("b2", [P, D // P], f32, kind="ExternalInput")
    xsT = nc.dram_tensor("xsT", [P, DT, TS], sdt, kind="ExternalInput")
    ws1 = nc.dram_tensor("ws1", [FS // WG, P, DT, WG], sdt, kind="ExternalInput")
    ws3 = nc.dram_tensor("ws3", [FS // WG, P, DT, WG], sdt, kind="ExternalInput")
    ws2 = nc.dram_tensor("ws2", [FS, D], sdt, kind="ExternalInput")
    bs1 = nc.dram_tensor("bs1", [P, FS // P], f32, kind="ExternalInput")
    bs3 = nc.dram_tensor("bs3", [P, FS // P], f32, kind="ExternalInput")
    bs2 = nc.dram_tensor("bs2", [P, D // P], f32, kind="ExternalInput")
    # spill quarter-job inputs
    xpT = nc.dram_tensor("xpT", [P, DT, CS], sdt, kind="ExternalInput")
    gws = nc.dram_tensor("gws", [1, CS], sdt, kind="ExternalInput")
    w1q = nc.dram_tensor("w1q", [P, DT, FH], sdt, kind="ExternalInput")
    w3q = nc.dram_tensor("w3q", [P, DT, FH], sdt, kind="ExternalInput")
    w2q = nc.dram_tensor("w2q", [P, FHT, D], sdt, kind="ExternalInput")
    b1q = nc.dram_tensor("b1q", [P, FHT], f32, kind="ExternalInput")
    b3q = nc.dram_tensor("b3q", [P, FHT], f32, kind="ExternalInput")
    b2q = nc.dram_tensor("b2q", [P, DT], f32, kind="ExternalInput")

    # ---- per-core outputs ----
    # bf16 outputs halve the store traffic; host combine upcasts to fp32.
    yT = nc.dram_tensor("yT", [D, CP], sdt, kind="ExternalOutput")
    ysT = nc.dram_tensor("ysT", [D, TS], sdt, kind="ExternalOutput")
    yqT = nc.dram_tensor("yqT", [D, CS], sdt, kind="ExternalOutput")

    CH = _chunks(CP)

    with tile.TileContext(nc) as tc:
        with (
            tc.tile_pool(name="tmp3", bufs=3) as htmp,
            tc.tile_pool(name="ps", bufs=4, space="PSUM") as psp,
        ):
            ytmp = htmp
            consts = tc.alloc_tile_pool(name="statics", bufs=1)
            # pools with bounded lifetimes, released imperatively so their
            # teardown overlaps later compute instead of serializing at the
            # kernel end:
            #  - xs + the ws1/ws3 stream die after the shared h-stage (their
            #    SBUF zone is reused by the spill-job weights)
            #  - ws2 stream + spill weights die before the expert path
            w2stream = tc.alloc_tile_pool(name="w2stream", bufs=4)
            earlyp = tc.alloc_tile_pool(name="early", bufs=1)
            wstream = tc.alloc_tile_pool(name="wstream", bufs=3)
            # ---------- expert-path prefetch plumbing ----------
            # issued in ~0.3-0.5MB chunks threaded through the shared-expert
            # loops on the scalar queue so the transfers never outrun the
            # latency-critical ws1/ws3/ws2 streams feeding the PE.
            b1_sb = consts.tile([P, FT], f32)
            b3_sb = consts.tile([P, FT], f32)
            b2_sb = consts.tile([P, DT], f32)
            gw_sb = consts.tile([P, CP], sdt, tag="gw")
            xg_sb = consts.tile([P, DT, CP], sdt, tag="xg")
            w1_sb = consts.tile([P, DT, F], sdt, tag="w1res")
            w3_sb = consts.tile([P, DT, F], sdt, tag="w3res")
            w2_sb = consts.tile([P, FT, D], sdt, tag="w2res")
            pf_chunks = []
            ws2_pre = []
            for fs in range(4):
                wb = w2stream.tile([P, D], sdt, tag="ws2b")
                pf_chunks.append(
                    (wb[:], ws2[:].rearrange("(o p) d -> p o d", p=P)[:, fs, :])
                )
                ws2_pre.append(wb)
            for d in range(DT):
                pf_chunks.append((xg_sb[:, d, :], xgT[:, d, :]))
            HF = F // 2
            for d in range(DT):
                for h in range(2):
                    pf_chunks.append(
                        (
                            w1_sb[:, d, h * HF : (h + 1) * HF],
                            w1[:, d, h * HF : (h + 1) * HF],
                        )
                    )
            for d in range(DT):
                for h in range(2):
                    pf_chunks.append(
                        (
                            w3_sb[:, d, h * HF : (h + 1) * HF],
                            w3[:, d, h * HF : (h + 1) * HF],
                        )
                    )
            pf_chunks.append((b1_sb[:], b1[:]))
            pf_chunks.append((b3_sb[:], b3[:]))
            pf_chunks.append((b2_sb[:], b2[:]))
            pf_chunks.append((gw_sb[:], gw[:].to_broadcast([P, CP])))
            for fi in range(FT):
                pf_chunks.append((w2_sb[:, fi, :], w2[:, fi, :]))
            pf_i = [0]

            def issue_prefetch(n=1):
                while n > 0 and pf_i[0] < len(pf_chunks):
                    dst, src = pf_chunks[pf_i[0]]
                    nc.scalar.dma_start(dst, src)
                    pf_i[0] += 1
                    n -= 1

            # a-buffer: holds a_shared [P, FST, 512] during the shared stage,
            # then two rotating [P, FT, 512] slabs for the expert chunks.
            ab = consts.tile([P, FST * 512], sdt, tag="abuf")
            a_shared = ab[:, : FST * 512].rearrange("p (f n) -> p f n", f=FST)

            def a_expert(n):
                off = (n % 2) * (FT * 512)
                return ab[:, off : off + FT * 512].rearrange("p (f n) -> p f n", f=FT)

            bs1_sb = consts.tile([P, FST], f32)
            bs3_sb = consts.tile([P, FST], f32)
            bs2_sb = consts.tile([P, DT], f32)


            # ---------- shared-expert inputs first (PE starts on these) ----------
            ws1_g0 = wstream.tile([P, DT, WG], sdt, tag="ws1g")
            ws3_g0 = wstream.tile([P, DT, WG], sdt, tag="ws3g")
            xs_sb = earlyp.tile([P, DT, TS], sdt, tag="xs")
            nc.sync.dma_start(ws1_g0[:, :, :P], ws1[0][:, :, :P])
            nc.gpsimd.dma_start(xs_sb[:, DT // 2 :, :], xsT[:, DT // 2 :, :])
            nc.scalar.dma_start(xs_sb[:, : DT // 2, :], xsT[:, : DT // 2, :])
            nc.sync.dma_start(ws1_g0[:, :, P:], ws1[0][:, :, P:])
            nc.gpsimd.dma_start(ws3_g0[:, :, :P], ws3[0][:, :, :P])
            nc.gpsimd.dma_start(ws3_g0[:, :, P:], ws3[0][:, :, P:])
            nc.scalar.dma_start(bs1_sb[:], bs1[:])
            nc.scalar.dma_start(bs3_sb[:], bs3[:])
            nc.scalar.dma_start(bs2_sb[:], bs2[:])

            # PE clock warmup: the HAM gate holds the PE at 1.2GHz until it
            # has been ~3.4us busy. Dummy matmuls (reading the zeroed head of
            # the a-buffer) cover the initial input-DMA wait so the real
            # matmuls start at 2.4GHz even when the DMA subsystem starts slow.
            nc.vector.memset(ab[:, :TS], 0)
            pwarm = psp.tile([P, TS], f32, tag="ph1", name="pwarm")
            N_WARM = 20
            for i in range(N_WARM):
                nc.tensor.matmul(
                    pwarm[:, : 2 * P],
                    lhsT=ab[:, :P],
                    rhs=ab[:, : 2 * P],
                    start=(i == 0),
                    stop=(i == N_WARM - 1),
                )

            # ---------- shared expert: h1s/h3s -> a_shared ----------
            for g in range(FST // 2):
                if g == 0:
                    ws1_g, ws3_g = ws1_g0, ws3_g0
                else:
                    ws1_g = wstream.tile([P, DT, WG], sdt, tag="ws1g")
                    nc.sync.dma_start(ws1_g[:], ws1[g])
                    ws3_g = wstream.tile([P, DT, WG], sdt, tag="ws3g")
                    nc.sync.dma_start(ws3_g[:], ws3[g])
                if g >= 2:
                    issue_prefetch(2)
                for sub in range(2):
                    fs = g * 2 + sub
                    scols = slice(sub * P, (sub + 1) * P)
                    ph1 = psp.tile([P, TS], f32, tag="ph1")
                    ph3 = psp.tile([P, TS], f32, tag="ph3")
                    for d in range(DT):
                        nc.tensor.matmul(
                            ph1[:],
                            lhsT=ws1_g[:, d, scols],
                            rhs=xs_sb[:, d, :],
                            start=(d == 0),
                            stop=(d == DT - 1),
                        )
                    for d in range(DT):
                        nc.tensor.matmul(
                            ph3[:],
                            lhsT=ws3_g[:, d, scols],
                            rhs=xs_sb[:, d, :],
                            start=(d == 0),
                            stop=(d == DT - 1),
                        )
                    h1 = htmp.tile([P, TS], f32, tag="h1")
                    nc.vector.tensor_scalar_add(h1[:], ph1[:], bs1_sb[:, fs : fs + 1])
                    prod = htmp.tile([P, TS], f32, tag="prod")
                    nc.vector.scalar_tensor_tensor(
                        prod[:],
                        in0=ph3[:],
                        scalar=bs3_sb[:, fs : fs + 1],
                        in1=h1[:],
                        op0=mybir.AluOpType.add,
                        op1=mybir.AluOpType.mult,
                    )
                    nc.scalar.activation(
                        a_shared[:, fs, :],
                        prod[:],
                        mybir.ActivationFunctionType.Silu,
                    )


            xp_sb = consts.tile([P, DT, CS], sdt, tag="xp")
            gws_sb = consts.tile([P, CS], sdt, tag="gws")
            b1q_sb = consts.tile([P, FHT], f32)
            b3q_sb = consts.tile([P, FHT], f32)
            b2q_sb = consts.tile([P, DT], f32)

            # ---------- shared expert: ys = a_shared @ Ws2 ----------
            # single pass over Ws2 (streamed once) accumulating all 8 d-tiles
            # in 8 PSUM banks at once
            pys = [
                psp.tile(
                    [P, TS], f32, tag=("ph1" if i < 4 else "ph3"), name=f"pys_{i}"
                )
                for i in range(DT)
            ]
            for fs in range(FST):
                if fs < len(ws2_pre):
                    ws2_b = ws2_pre[fs]
                else:
                    ws2_b = w2stream.tile([P, D], sdt, tag="ws2b")
                    nc.sync.dma_start(
                        ws2_b[:],
                        ws2[:].rearrange("(o p) d -> p o d", p=P)[:, fs, :],
                    )
                if fs == 2:
                    # Release xs + the ws1/ws3 stream HERE (not at the h/y
                    # boundary): the release is a cross-engine rendezvous,
                    # and by fs=2 the vector/scalar shared-h tail has
                    # drained, so the tensor queue arrives last and never
                    # stalls. The spill weights then reuse the freed zone.
                    wstream.release()
                    earlyp.release()
                    spillw = tc.alloc_tile_pool(name="spillw", bufs=1)
                    w1q_sb = spillw.tile([P, DT, FH], sdt, tag="w1q")
                    w3q_sb = spillw.tile([P, DT, FH], sdt, tag="w3q")
                    w2q_sb = spillw.tile([P, FHT, D], sdt, tag="w2q")
                    spill_chunks = []
                    for d in range(0, DT, 2):
                        spill_chunks.append(
                            (w1q_sb[:, d : d + 2, :], w1q[:, d : d + 2, :])
                        )
                    for d in range(0, DT, 2):
                        spill_chunks.append(
                            (w3q_sb[:, d : d + 2, :], w3q[:, d : d + 2, :])
                        )
                    for fi in range(FHT):
                        spill_chunks.append((w2q_sb[:, fi, :], w2q[:, fi, :]))
                    spill_chunks.append((xp_sb[:], xpT[:]))
                    spill_chunks.append((gws_sb[:], gws[:].to_broadcast([P, CS])))
                    spill_chunks.append((b1q_sb[:], b1q[:]))
                    spill_chunks.append((b3q_sb[:], b3q[:]))
                    spill_chunks.append((b2q_sb[:], b2q[:]))
                    # ahead of the primary-w2 chunks: the spill stage runs
                    # right after shared-y, while primary w2 isn't read until
                    # ~60us into the expert path
                    ins_at = max(48, pf_i[0])
                    pf_chunks[ins_at:ins_at] = spill_chunks
                issue_prefetch(2)
                for d in range(DT):
                    nc.tensor.matmul(
                        pys[d][:],
                        lhsT=ws2_b[:, d * P : (d + 1) * P],
                        rhs=a_shared[:, fs, :],
                        start=(fs == 0),
                        stop=(fs == FST - 1),
                    )
            for dp in range(DT // 2):
                dA = 2 * dp
                yo2 = ytmp.tile([P, 2, TS], sdt, tag="yo2")
                nc.vector.tensor_scalar_add(
                    yo2[:, 0, :], pys[dA][:], bs2_sb[:, dA : dA + 1]
                )
                nc.vector.tensor_scalar_add(
                    yo2[:, 1, :], pys[dA + 1][:], bs2_sb[:, dA + 1 : dA + 2]
                )
                nc.gpsimd.dma_start(
                    ysT[:].rearrange("(o p) c -> p o c", p=P)[:, dA : dA + 2, :],
                    yo2[:],
                )



            # flush any prefetch chunks the shared loops didn't cover
            issue_prefetch(len(pf_chunks))

            # ---------- spill quarter-job: deferred emission ----------
            # each step is emitted interleaved into the expert chunk-0
            # h-stage so the spill's vector/scalar chain (which dominates its
            # tiny 64-col matmuls) hides under the expert matmuls
            aq = consts.tile([P, FHT, CS], sdt, tag="aq")

            def spill_h_step(f):
                ph1 = psp.tile([P, TS], f32, tag="ph1")
                ph3 = psp.tile([P, TS], f32, tag="ph3")
                fcols = slice(f * P, (f + 1) * P)
                for d in range(DT):
                    nc.tensor.matmul(
                        ph1[:, :CS],
                        lhsT=w1q_sb[:, d, fcols],
                        rhs=xp_sb[:, d, :],
                        start=(d == 0),
                        stop=(d == DT - 1),
                    )
                for d in range(DT):
                    nc.tensor.matmul(
                        ph3[:, :CS],
                        lhsT=w3q_sb[:, d, fcols],
                        rhs=xp_sb[:, d, :],
                        start=(d == 0),
                        stop=(d == DT - 1),
                    )
                h1 = htmp.tile([P, TS], f32, tag="h1")
                nc.vector.tensor_scalar_add(
                    h1[:, :CS], ph1[:, :CS], b1q_sb[:, f : f + 1]
                )
                prod = htmp.tile([P, TS], f32, tag="prod")
                nc.vector.scalar_tensor_tensor(
                    prod[:, :CS],
                    in0=ph3[:, :CS],
                    scalar=b3q_sb[:, f : f + 1],
                    in1=h1[:, :CS],
                    op0=mybir.AluOpType.add,
                    op1=mybir.AluOpType.mult,
                )
                nc.scalar.activation(
                    aq[:, f, :], prod[:, :CS], mybir.ActivationFunctionType.Silu
                )

            def spill_y_step(dp):
                dA, dB = 2 * dp, 2 * dp + 1
                pyA = psp.tile([P, TS], f32, tag="ph1")
                pyB = psp.tile([P, TS], f32, tag="ph3")
                for f in range(FHT):
                    nc.tensor.matmul(
                        pyA[:, :CS],
                        lhsT=w2q_sb[:, f, dA * P : (dA + 1) * P],
                        rhs=aq[:, f, :],
                        start=(f == 0),
                        stop=(f == FHT - 1),
                    )
                    nc.tensor.matmul(
                        pyB[:, :CS],
                        lhsT=w2q_sb[:, f, dB * P : (dB + 1) * P],
                        rhs=aq[:, f, :],
                        start=(f == 0),
                        stop=(f == FHT - 1),
                    )
                yo2 = ytmp.tile([P, 2, TS], sdt, tag="yo2")
                for k, (d, py) in enumerate(((dA, pyA), (dB, pyB))):
                    nc.vector.scalar_tensor_tensor(
                        yo2[:, k, :CS],
                        in0=py[:, :CS],
                        scalar=b2q_sb[:, d : d + 1],
                        in1=gws_sb[:, :],
                        op0=mybir.AluOpType.add,
                        op1=mybir.AluOpType.mult,
                    )
                eng = nc.sync if dp % 2 == 0 else nc.gpsimd
                eng.dma_start(
                    yqT[:].rearrange("(o p) c -> p o c", p=P)[:, dA : dB + 1, :],
                    yo2[:, :, :CS],
                )

            spill_steps = [lambda f=f: spill_h_step(f) for f in range(FHT)]
            spill_steps += [lambda dp=dp: spill_y_step(dp) for dp in range(DT // 2)]

            # ---------- expert path ----------
            for n, (cs_, cw) in enumerate(CH):
                a_n = a_expert(n)
                ncols = slice(cs_, cs_ + cw)
                for f in range(FT):
                    ph1 = psp.tile([P, TS], f32, tag="ph1")
                    ph3 = psp.tile([P, TS], f32, tag="ph3")
                    fcols = slice(f * P, (f + 1) * P)
                    for d in range(DT):
                        nc.tensor.matmul(
                            ph1[:, :cw],
                            lhsT=w1_sb[:, d, fcols],
                            rhs=xg_sb[:, d, ncols],
                            start=(d == 0),
                            stop=(d == DT - 1),
                        )
                    for d in range(DT):
                        nc.tensor.matmul(
                            ph3[:, :cw],
                            lhsT=w3_sb[:, d, fcols],
                            rhs=xg_sb[:, d, ncols],
                            start=(d == 0),
                            stop=(d == DT - 1),
                        )
                    h1 = htmp.tile([P, TS], f32, tag="h1")
                    nc.vector.tensor_scalar_add(
                        h1[:, :cw], ph1[:, :cw], b1_sb[:, f : f + 1]
                    )
                    prod = htmp.tile([P, TS], f32, tag="prod")
                    nc.vector.scalar_tensor_tensor(
                        prod[:, :cw],
                        in0=ph3[:, :cw],
                        scalar=b3_sb[:, f : f + 1],
                        in1=h1[:, :cw],
                        op0=mybir.AluOpType.add,
                        op1=mybir.AluOpType.mult,
                    )
                    nc.scalar.activation(
                        a_n[:, f, :cw],
                        prod[:, :cw],
                        mybir.ActivationFunctionType.Silu,
                    )
                    if f % 2 == 1 and spill_steps:
                        spill_steps.pop(0)()
                    if n == 0:
                        issue_prefetch()
                if n == 0:
                    spillw.release()
                    w2stream.release()
                for dp in range(DT // 2):
                    dA, dB = 2 * dp, 2 * dp + 1
                    pyA = psp.tile([P, TS], f32, tag="ph1")
                    pyB = psp.tile([P, TS], f32, tag="ph3")
                    for f in range(FT):
                        nc.tensor.matmul(
                            pyA[:, :cw],
                            lhsT=w2_sb[:, f, dA * P : (dA + 1) * P],
                            rhs=a_n[:, f, :cw],
                            start=(f == 0),
                            stop=(f == FT - 1),
                        )
                        nc.tensor.matmul(
                            pyB[:, :cw],
                            lhsT=w2_sb[:, f, dB * P : (dB + 1) * P],
                            rhs=a_n[:, f, :cw],
                            start=(f == 0),
                            stop=(f == FT - 1),
                        )
                    yo2 = ytmp.tile([P, 2, TS], sdt, tag="yo2")
                    for k, (d, py) in enumerate(((dA, pyA), (dB, pyB))):
                        nc.vector.scalar_tensor_tensor(
                            yo2[:, k, :cw],
                            in0=py[:, :cw],
                            scalar=b2_sb[:, d : d + 1],
                            in1=gw_sb[:, ncols],
                            op0=mybir.AluOpType.add,
                            op1=mybir.AluOpType.mult,
                        )
                    eng = nc.sync if dp % 2 == 0 else nc.gpsimd
                    eng.dma_start(
                        yT[:].rearrange("(o p) c -> p o c", p=P)[
                            :, dA : dB + 1, ncols
                        ],
                        yo2[:, :, :cw],
                    )
            consts.release()

    nc.compile()
    return nc


def _get_program(CP, cfg):
    key = (CP, cfg)
    if key not in _COMPILED:
        _COMPILED[key] = build_program(CP, cfg)
    return _COMPILED[key]


def _pack_bias(b):
    """[K] -> [128, K/128] partition-major (element (p, o) = b[o*128+p])."""
    b = np.asarray(b, dtype=np.float32)
    return np.ascontiguousarray(b.reshape(-1, P).T)


def _route(xf, Wg):
    """Host gating: softmax -> top-2 -> renormalized weights (float64)."""
    logits = xf.astype(np.float64) @ Wg.astype(np.float64)
    m = logits.max(-1, keepdims=True)
    p = np.exp(logits - m)
    scores = p / p.sum(-1, keepdims=True)
    eidx = np.argsort(-scores, axis=-1, kind="stable")[:, :TOPK]
    sel = np.take_along_axis(scores, eidx, -1)
    sm = sel.max(-1, keepdims=True)
    pe = np.exp(sel - sm)
    ew = pe / pe.sum(-1, keepdims=True)
    return eidx, ew.astype(np.float32)


def prepare_in_maps(x, Wg, W1, b1, W3, b3, W2, b2, Ws1, bs1, Ws3, bs3, Ws2, bs2, cfg=MM_CFG):
    xf = np.ascontiguousarray(np.asarray(x, dtype=np.float32).reshape(-1, D))
    eidx, ew = _route(xf, np.asarray(Wg, dtype=np.float32))

    flat_e = eidx.reshape(-1)
    flat_w = ew.reshape(-1)
    tok = np.repeat(np.arange(T), TOPK)
    order = np.argsort(flat_e, kind="stable")
    se, st, sw = flat_e[order], tok[order], flat_w[order]
    counts = np.bincount(se, minlength=E)
    offs = np.concatenate([[0], np.cumsum(counts)])
    idx_lists = [st[offs[e] : offs[e + 1]] for e in range(E)]
    gw_lists = [sw[offs[e] : offs[e + 1]] for e in range(E)]

    CP, jobs = plan_spill(counts)

    np_mdt = _np_mm_dtype(cfg)

    def tile_kxn(a, K):
        # [K, N] -> [P, K/P, N] partition-major
        a = np.asarray(a, dtype=np.float32)
        return a.reshape(K // P, P, -1).transpose(1, 0, 2).astype(np_mdt)

    # shared-expert weights are identical on every core: pack once
    ws1_t = np.asarray(Ws1, dtype=np.float32).reshape(D // P, P, FS // WG, WG)
    ws1_t = ws1_t.transpose(2, 1, 0, 3).astype(np_mdt)
    ws3_t = np.asarray(Ws3, dtype=np.float32).reshape(D // P, P, FS // WG, WG)
    ws3_t = ws3_t.transpose(2, 1, 0, 3).astype(np_mdt)
    ws2_t = np.asarray(Ws2, dtype=np.float32).astype(np_mdt)
    bs1_p, bs3_p, bs2_p = _pack_bias(bs1), _pack_bias(bs3), _pack_bias(bs2)

    W1 = np.asarray(W1, dtype=np.float32)
    W3 = np.asarray(W3, dtype=np.float32)
    W2 = np.asarray(W2, dtype=np.float32)
    b1 = np.asarray(b1, dtype=np.float32)
    b3 = np.asarray(b3, dtype=np.float32)
    b2 = np.asarray(b2, dtype=np.float32)

    in_maps = []
    for c in range(E):
        cnt = min(int(counts[c]), CP)
        xg = np.zeros((CP, D), dtype=np.float32)
        xg[:cnt] = xf[idx_lists[c][:cnt]]
        gwv = np.zeros((1, CP), dtype=np.float32)
        gwv[0, :cnt] = gw_lists[c][:cnt]
        xsl = xf[c * TS : (c + 1) * TS]
        # spill quarter-job for this core
        xp = np.zeros((CS, D), dtype=np.float32)
        gqv = np.zeros((1, CS), dtype=np.float32)
        if c < len(jobs):
            e, q, s, w = jobs[c]
            xp[:w] = xf[idx_lists[e][s : s + w]]
            gqv[0, :w] = gw_lists[e][s : s + w]
            w1q_h = W1[e][:, q * FH : (q + 1) * FH]
            w3q_h = W3[e][:, q * FH : (q + 1) * FH]
            w2q_h = W2[e][q * FH : (q + 1) * FH, :]
            b1q_h = b1[e][q * FH : (q + 1) * FH]
            b3q_h = b3[e][q * FH : (q + 1) * FH]
            b2q_h = b2[e] if q == 0 else np.zeros_like(b2[e])
        else:
            w1q_h = np.zeros((D, FH), dtype=np.float32)
            w3q_h = np.zeros((D, FH), dtype=np.float32)
            w2q_h = np.zeros((FH, D), dtype=np.float32)
            b1q_h = np.zeros((FH,), dtype=np.float32)
            b3q_h = np.zeros((FH,), dtype=np.float32)
            b2q_h = np.zeros((D,), dtype=np.float32)
        in_maps.append(
            {
                "xgT": tile_kxn(xg.T, D),
                "gw": gwv.astype(np_mdt),
                "w1": tile_kxn(W1[c], D),
                "w3": tile_kxn(W3[c], D),
                "w2": tile_kxn(W2[c], F),
                "b1": _pack_bias(b1[c]),
                "b3": _pack_bias(b3[c]),
                "b2": _pack_bias(b2[c]),
                "xsT": tile_kxn(xsl.T, D),
                "ws1": ws1_t,
                "ws3": ws3_t,
                "ws2": ws2_t,
                "bs1": bs1_p,
                "bs3": bs3_p,
                "bs2": bs2_p,
                "xpT": tile_kxn(xp.T, D),
                "gws": gqv.astype(np_mdt),
                "w1q": tile_kxn(w1q_h, D),
                "w3q": tile_kxn(w3q_h, D),
                "w2q": tile_kxn(w2q_h, FH),
                "b1q": _pack_bias(b1q_h),
                "b3q": _pack_bias(b3q_h),
                "b2q": _pack_bias(b2q_h),
            }
        )
    return in_maps, idx_lists, counts, CP, jobs


def combine(results, idx_lists, counts, CP, jobs, x_shape, x_dtype):
    y = np.empty((D, T), dtype=np.float32)
    for c in range(E):
        y[:, c * TS : (c + 1) * TS] = np.asarray(results[c]["ysT"], dtype=np.float32)
    for c in range(E):
        cnt = min(int(counts[c]), CP)
        if cnt:
            cols = idx_lists[c][:cnt]
            y[:, cols] += np.asarray(results[c]["yT"][:, :cnt], dtype=np.float32)
    for c, job in enumerate(jobs):
        e, q, s, w = job
        cols = idx_lists[e][s : s + w]
        y[:, cols] += np.asarray(results[c]["yqT"][:, :w], dtype=np.float32)
    return np.ascontiguousarray(y.T).reshape(x_shape).astype(x_dtype, copy=False)


def run(x, Wg, W1, b1, W3, b3, W2, b2, Ws1, bs1, Ws3, bs3, Ws2, bs2,
        cfg=MM_CFG, trace=False, trace_kwargs=None):
    in_maps, idx_lists, counts, CP, jobs = prepare_in_maps(
        x, Wg, W1, b1, W3, b3, W2, b2, Ws1, bs1, Ws3, bs3, Ws2, bs2, cfg
    )
    nc = _get_program(CP, cfg)
    res = run_bass_kernel_spmd(
        nc, in_maps, list(range(E)), trace=trace, **(trace_kwargs or {})
    )
    out = combine(
        res.results, idx_lists, counts, CP, jobs,
        np.asarray(x).shape, np.asarray(x).dtype,
    )
    return out, res


def kernel(**inputs):
    out, _ = run(**inputs)
    return out
